# revision 16
# baseline (speedup 1.0000x reference)
"""BERT layer kernel for 8 TRN2 NeuronCores.

Sharding: 8 cores = 4 batch elements x 2 query-halves (1024 queries each).
Each core is fully independent (no collectives): it computes attention for
its 1024 queries against its batch element's key set, then proj/LN1/FFN/LN2
for its tokens.

Key ideas:
- Mask compaction on CPU: ~50% of keys have mask=0 and contribute exp(-100)
  ~= 0; only unmasked keys (padded to a multiple of 128) are shipped/computed.
- Feature-major (transposed) activations so every matmul contracts on the
  partition dim with zero on-device transposes of x (CPU pre-transposes).
- Input DMAs issue on one FIFO HWDGE ring in strictly critical-first order
  (xkvT/Wv/Wq/xqT/Wk first, FFN weights last); the V ones-column is a
  device-side memset (padded keys contribute exp(-100) ~ 0 to the
  denominator, so all-ones is exact) -- a DMA of it would emit 9216 2-byte
  descriptors and clog the ring for ~11us.
- Scores computed key-major (sT[k, q]); softmax numerator exp(0.125*s + bias)
  runs on ScalarE directly from PSUM with the mask as a per-partition bias.
- A ones-column appended to V makes the PV matmul emit softmax denominators
  for free; each head-pair is normalized immediately after its PV completes
  (reciprocal on 2 partitions + DMA partition-broadcast + one DVE multiply),
  fully overlapped with the next pair's projections/attention.
- proj and FFN2 accumulate token-major PSUM tiles (lhsT = activations), so
  LN1/LN2 read PSUM directly: no LN transposes, no attT/ffT intermediates.
  Residual adds (x, h) and the b2 row ride the same PSUM accumulation as
  identity / ones-outer-product matmuls.
- LN rsqrt(var+eps) via fast-inverse-sqrt (bit trick + 1 Newton step) on the
  DVE: keeps SQRT off ScalarE so only EXP and GELU activation tables load
  (table switches cost ~1.3us each and serialize the ACT pipeline).
- FFN2 runs per query-tile (gact for the whole half staged in SBUF), so each
  tile's LN2 overlaps the next tile's matmuls; only the last tile's chain is
  exposed.
- All matmuls fp16 (full PE rate, ~1.5e-4 rounding).
"""
import os
import sys

for _p in ("/opt/trn_rl_repo", "/root/.axon_site/_ro/trn_rl_repo"):
    if os.path.isdir(_p) and _p not in sys.path:
        sys.path.append(_p)

import numpy as np
import concourse.bacc as bacc
import concourse.tile as tile
from concourse import mybir
from concourse.bass_utils import run_bass_kernel_spmd
from concourse.masks import make_identity

P = 128
B, S, D = 4, 2048, 512
H, DK, DV = 8, 64, 64
DFF = 2048
NQ = 1024          # queries per core
QT_TILES = NQ // P  # 8
DC = D // P         # 4 feature chunks
FC = DFF // P       # 16 ffn chunks
LN_EPS = 1e-5
RSQRT_MAGIC = 0x5F3759DF

F32 = mybir.dt.float32
F16 = mybir.dt.float16
I32 = mybir.dt.int32
AF = mybir.ActivationFunctionType
ALU = mybir.AluOpType

_nc_cache = {}
last_exec_ns = None
last_trace_path = None
last_results = None

# rb: 'gpsimd' = partition_broadcast custom op; 'matmul' = K=2 PE broadcast
RB_MODE = os.environ.get("BERT_RB_MODE", "gpsimd")
# rsqrt: 'dve' = fast-inverse-sqrt bit trick; 'scalar' = ACT sqrt + DVE recip
RSQRT_MODE = os.environ.get("BERT_RSQRT_MODE", "dve")


def _build(NK):
    """Build the per-core Bass program for NK (padded, multiple of 128) keys."""
    KT = NK // P
    nc = bacc.Bacc(None, target_bir_lowering=False)

    # ---- DRAM I/O ----
    xqT_d = nc.dram_tensor("xqT", [D, NQ], F16, kind="ExternalInput")
    xkvT_d = nc.dram_tensor("xkvT", [D, NK], F16, kind="ExternalInput")
    xres_d = nc.dram_tensor("xres", [NQ, D], F16, kind="ExternalInput")
    mb_d = nc.dram_tensor("maskbias", [P, KT], F32, kind="ExternalInput")
    WqT_d = nc.dram_tensor("WqT", [D, D], F16, kind="ExternalInput")
    WkT_d = nc.dram_tensor("WkT", [D, D], F16, kind="ExternalInput")
    WvT_d = nc.dram_tensor("WvT", [D, D], F16, kind="ExternalInput")
    WpT_d = nc.dram_tensor("WpT", [D, D], F16, kind="ExternalInput")
    W1T_d = nc.dram_tensor("W1T", [D, DFF], F16, kind="ExternalInput")
    W2T_d = nc.dram_tensor("W2T", [DFF, D], F16, kind="ExternalInput")
    b1_d = nc.dram_tensor("b1r", [P, FC], F32, kind="ExternalInput")
    b2_d = nc.dram_tensor("b2r", [1, D], F16, kind="ExternalInput")
    out_d = nc.dram_tensor("out", [NQ, D], F32, kind="ExternalOutput")

    with tile.TileContext(nc) as tc:
        # Pools close LIFO; opened in reverse order of tensor death.
        pp_cm = tc.tile_pool(name="pp", bufs=1)
        pAB_cm = tc.tile_pool(name="pAB", bufs=1)   # ctxT + WpT
        pA_cm = tc.tile_pool(name="pA", bufs=1)     # QT/KTs/Vs
        ph1_cm = tc.tile_pool(name="ph1", bufs=1)   # xqT/xkvT/Wq/Wk/Wv
        pW_cm = tc.tile_pool(name="pW", bufs=1, side="right")
        pp = pp_cm.__enter__()
        pW = pW_cm.__enter__()
        pAB = pAB_cm.__enter__()
        pA = pA_cm.__enter__()
        ph1 = ph1_cm.__enter__()

        # tiles (declared in dependency-phase order)
        mb_sb = pp.tile([P, KT], F32, tag="mb")
        b1_sb = pp.tile([P, FC], F32, tag="b1")
        b2_sb = pp.tile([1, D], F16, tag="b2")
        magic = pp.tile([P, 1], I32, tag="magic")
        onesr = pp.tile([1, P], F16, tag="onesr")
        eps_sb = pp.tile([P, 1], F32, tag="eps")
        ident = pp.tile([P, P], F32, tag="ident")
        ident16 = pp.tile([P, P], F16, tag="ident16")
        WpT = pAB.tile([P, DC, D], F16, tag="WpT")
        QT = pA.tile([P, DC, NQ], F16, tag="QT")
        KTs = pA.tile([P, DC, NK], F16, tag="KTs")
        Vs = pA.tile([P, KT, H, DV + 1], F16, tag="Vs")
        ctxT = pAB.tile([P, DC, NQ], F16, tag="ctxT")
        xqT = ph1.tile([P, DC, NQ], F16, tag="xqT")
        xkvT = ph1.tile([P, DC, NK], F16, tag="xkvT")
        Wq = ph1.tile([P, DC, D], F16, tag="Wq")
        Wk = ph1.tile([P, DC, D], F16, tag="Wk")
        Wv = ph1.tile([P, DC, D], F16, tag="Wv")
        W1 = pW.tile([P, DC, DFF], F16, tag="W1")
        W2 = pW.tile([P, FC, D], F16, tag="W2")
        xres = pW.tile([P, QT_TILES, D], F16, tag="xres")
        h_sb = pp.tile([P, QT_TILES, D], F16, tag="h_sb")
        hT = pp.tile([P, DC, NQ], F16, tag="hT")

        # input DMAs: one FIFO ring, critical-first
        nc.sync.dma_start(mb_sb[:], mb_d[:])
        nc.sync.dma_start(xkvT[:], xkvT_d.rearrange("(c p) k -> p c k", p=P))
        nc.sync.dma_start(Wv[:], WvT_d.rearrange("(c p) d -> p c d", p=P))
        nc.sync.dma_start(Wq[:], WqT_d.rearrange("(c p) d -> p c d", p=P))
        nc.sync.dma_start(xqT[:], xqT_d.rearrange("(c p) q -> p c q", p=P))
        nc.sync.dma_start(Wk[:], WkT_d.rearrange("(c p) d -> p c d", p=P))
        nc.sync.dma_start(WpT[:], WpT_d.rearrange("(c p) d -> p c d", p=P))
        nc.sync.dma_start(b1_sb[:], b1_d[:])
        nc.sync.dma_start(b2_sb[:], b2_d[:])
        nc.sync.dma_start(W1[:], W1T_d.rearrange("(c p) f -> p c f", p=P))
        nc.sync.dma_start(W2[:], W2T_d.rearrange("(c p) d -> p c d", p=P))
        nc.sync.dma_start(xres[:], xres_d.rearrange("(t p) d -> p t d", p=P))

        nc.vector.memset(Vs[:, :, :, DV], 1.0)  # softmax-denominator column
        nc.vector.memset(magic[:], RSQRT_MAGIC)
        nc.vector.memset(onesr[:], 1.0)
        nc.vector.memset(eps_sb[:], LN_EPS)
        make_identity(nc, ident[:])
        nc.vector.tensor_copy(ident16[:], ident[:])

        kchunks = ([(i * 384, 384) for i in range(NK // 384)]
                   if NK % 384 == 0 else
                   [(s0, min(512, NK - s0)) for s0 in range(0, NK, 512)])
        with (
            tc.tile_pool(name="ps1", bufs=2, space="PSUM") as ps1,
            tc.tile_pool(name="epool", bufs=3) as epool,
            tc.tile_pool(name="dstp", bufs=2) as dstp,
            tc.tile_pool(name="denp", bufs=2) as denp,
            tc.tile_pool(name="rbp", bufs=2) as rbp,
            tc.tile_pool(name="psS", bufs=2, space="PSUM") as psS,
            tc.tile_pool(name="psC", bufs=1, space="PSUM") as psC,
        ):
            # V[k, dv] token-major (+ ones col memset above) -- feeds all pairs
            for kt in range(KT):
                ps = ps1.tile([P, D], F32, tag="p1", name="psv")
                for c in range(DC):
                    nc.tensor.matmul(
                        ps[:], lhsT=xkvT[:, c, kt * P:(kt + 1) * P],
                        rhs=Wv[:, c, :],
                        start=(c == 0), stop=(c == DC - 1))
                nc.vector.tensor_copy(
                    Vs[:, kt, :, 0:DV],
                    ps.rearrange("p (h v) -> p h v", h=H))
            # per head-pair: project QT/KT chunk, then attention for the pair.
            # The dense K=128 projection matmuls of pair c+1 fill the PE while
            # ScalarE chews pair c's exps (keeps HAM warm).
            for c in range(DC):
                ha, hb = 2 * c, 2 * c + 1
                for qn in range(NQ // 512):
                    ps = ps1.tile([P, 512], F32, tag="p1", name="psq")
                    for cc in range(DC):
                        nc.tensor.matmul(
                            ps[:],
                            lhsT=Wq[:, cc, c * P:(c + 1) * P],
                            rhs=xqT[:, cc, qn * 512:(qn + 1) * 512],
                            start=(cc == 0), stop=(cc == DC - 1))
                    nc.vector.tensor_copy(
                        QT[:, c, qn * 512:(qn + 1) * 512], ps[:])
                for (s0, w) in kchunks:
                    ps = ps1.tile([P, 512], F32, tag="p1", name="psk")
                    for cc in range(DC):
                        nc.tensor.matmul(
                            ps[:, 0:w],
                            lhsT=Wk[:, cc, c * P:(c + 1) * P],
                            rhs=xkvT[:, cc, s0:s0 + w],
                            start=(cc == 0), stop=(cc == DC - 1))
                    nc.vector.tensor_copy(KTs[:, c, s0:s0 + w], ps[:, 0:w])
                # attention for heads (ha, hb): row-packed K=64 matmuls into
                # one shared psum tile (a: cols 0:512, b: 512:1024)
                den_ab = [denp.tile([1, NQ], F32, tag="dena", name="den_a"),
                          denp.tile([1, NQ], F32, tag="denb", name="den_b")]
                for qn in range(2):
                    ctxa = psC.tile([P, 512], F32, tag="ctxa", name="ctxa")
                    ctxb = psC.tile([P, 512], F32, tag="ctxb", name="ctxb")
                    for kt in range(KT):
                        sp = psS.tile([P, 1024], F32, tag="sT", name="sp")
                        nc.tensor.matmul(
                            sp[:, 0:512],
                            lhsT=KTs[0:64, c, kt * P:(kt + 1) * P],
                            rhs=QT[0:64, c, qn * 512:(qn + 1) * 512],
                            start=True, stop=True)
                        nc.tensor.matmul(
                            sp[:, 512:1024],
                            lhsT=KTs[64:128, c, kt * P:(kt + 1) * P],
                            rhs=QT[64:128, c, qn * 512:(qn + 1) * 512],
                            start=True, stop=True)
                        e_t = epool.tile([P, 1024], F16, tag="E", name="e_t")
                        nc.scalar.activation(e_t[:], sp[:], AF.Exp,
                                             bias=mb_sb[:, kt:kt + 1],
                                             scale=float(DK) ** -0.5)
                        nc.tensor.matmul(
                            ctxa[0:DV + 1, :], lhsT=Vs[:, kt, ha, :],
                            rhs=e_t[:, 0:512],
                            start=(kt == 0), stop=(kt == KT - 1))
                        nc.tensor.matmul(
                            ctxb[0:DV + 1, :], lhsT=Vs[:, kt, hb, :],
                            rhs=e_t[:, 512:1024],
                            start=(kt == 0), stop=(kt == KT - 1))
                    for (i, cx) in ((0, ctxa), (1, ctxb)):
                        dstage = dstp.tile([1, 512], F32, tag="dst",
                                           name="dstage")
                        nc.vector.tensor_copy(dstage[:], cx[DV:DV + 1, :])
                        nc.gpsimd.dma_start(
                            den_ab[i][0:1, qn * 512:(qn + 1) * 512],
                            dstage[:])
                        nc.vector.tensor_copy(
                            ctxT[i * 64:i * 64 + 64, c,
                                 qn * 512:(qn + 1) * 512],
                            cx[0:DV, :])
                # normalize this pair now; overlaps pair c+1's projections.
                # partition_broadcast requires its source on partition 0, so
                # each head's reciprocal goes to its own 1-partition tile.
                rcp_a = denp.tile([1, NQ], F16, tag="rcpa", name="rcp_a")
                rcp_b = denp.tile([1, NQ], F16, tag="rcpb", name="rcp_b")
                with nc.allow_low_precision(reason="fp16 softmax recip"):
                    nc.vector.reciprocal(rcp_a[:], den_ab[0][:])
                    nc.vector.reciprocal(rcp_b[:], den_ab[1][:])
                rb = rbp.tile([P, NQ], F16, tag="rb", name="rb")
                if RB_MODE == "gpsimd":
                    nc.gpsimd.partition_broadcast(rb[0:64, :], rcp_a[:])
                    nc.gpsimd.partition_broadcast(rb[64:128, :], rcp_b[:])
                else:
                    # ones-outer-product broadcast on the PE (K=1 matmuls)
                    for qn in range(2):
                        qs = slice(qn * 512, (qn + 1) * 512)
                        rp = ps1.tile([P, 512], F32, tag="p1", name="rb_ps")
                        nc.tensor.matmul(rp[0:64, :], lhsT=onesr[0:1, 0:64],
                                         rhs=rcp_a[0:1, qs],
                                         start=True, stop=True)
                        nc.tensor.matmul(rp[64:128, :], lhsT=onesr[0:1, 0:64],
                                         rhs=rcp_b[0:1, qs],
                                         start=True, stop=True)
                        nc.vector.tensor_copy(rb[:, qs], rp[:])
                nc.vector.tensor_mul(ctxT[:, c, :], ctxT[:, c, :], rb[:])
        ph1_cm.__exit__(None, None, None)
        pA_cm.__exit__(None, None, None)  # QT/KTs/Vs dead

        # ---------------- proj + LN1 (token-major, from PSUM) --------------
        def rsqrt_dve(pool, var_ap):
            """rstd = 1/sqrt(var+eps), fast-inverse-sqrt on DVE (no ACT
            table): y0 = bits(magic - bits(v)>>1); 1 Newton step."""
            if RSQRT_MODE == "scalar":
                w = pool.tile([P, 1], F32, tag="w", name="w")
                nc.scalar.activation(out=w[:], in_=var_ap, func=AF.Sqrt,
                                     bias=eps_sb[:, 0:1])
                nc.vector.reciprocal(out=w[:], in_=w[:])
                return w
            v = pool.tile([P, 1], F32, tag="v", name="v")
            nc.vector.tensor_scalar(out=v[:], in0=var_ap, scalar1=LN_EPS,
                                    scalar2=None, op0=ALU.add)
            y0 = pool.tile([P, 1], I32, tag="y0", name="y0")
            nc.vector.tensor_scalar(out=y0[:], in0=v.bitcast(I32),
                                    scalar1=1, scalar2=None,
                                    op0=ALU.logical_shift_right)
            nc.vector.tensor_tensor(out=y0[:], in0=magic[:], in1=y0[:],
                                    op=ALU.subtract)
            y0f = y0.bitcast(F32)
            w = pool.tile([P, 1], F32, tag="w", name="w")
            nc.vector.tensor_tensor(out=w[:], in0=y0f, in1=y0f, op=ALU.mult)
            nc.vector.tensor_scalar(out=w[:], in0=w[:], scalar1=v[:, 0:1],
                                    scalar2=-0.5, op0=ALU.mult, op1=ALU.mult)
            nc.vector.tensor_scalar(out=w[:], in0=w[:], scalar1=1.5,
                                    scalar2=None, op0=ALU.add)
            nc.vector.tensor_tensor(out=w[:], in0=y0f, in1=w[:], op=ALU.mult)
            return w

        with (
            tc.tile_pool(name="psP", bufs=2, space="PSUM") as psP,
            tc.tile_pool(name="ln1p", bufs=3, side="right") as ln1p,
        ):
            for qt in range(QT_TILES):
                ps = psP.tile([P, D], F32, tag="att", name="att_ps")
                for c in range(DC):
                    nc.tensor.matmul(
                        ps[:], lhsT=ctxT[:, c, qt * P:(qt + 1) * P],
                        rhs=WpT[:, c, :],
                        start=(c == 0), stop=False)
                nc.tensor.matmul(  # += I.T @ xres: the residual add
                    ps[:], lhsT=ident16[:], rhs=xres[:, qt, :],
                    start=False, stop=True)
                stats = ln1p.tile([P, nc.vector.BN_STATS_DIM], F32,
                                  tag="st1", name="stats")
                nc.vector.bn_stats(out=stats[:], in_=ps[:])
                mv = ln1p.tile([P, nc.vector.BN_AGGR_DIM], F32, tag="mv1",
                               name="mv")
                nc.vector.bn_aggr(out=mv[:], in_=stats[:])
                rstd = rsqrt_dve(ln1p, mv[:, 1:2])
                nc.vector.tensor_scalar(
                    out=h_sb[:, qt, :], in0=ps[:],
                    scalar1=mv[:, 0:1], scalar2=rstd[:, 0:1],
                    op0=ALU.subtract, op1=ALU.mult)
        pAB_cm.__exit__(None, None, None)  # ctxT, WpT dead

        # ---------------- h^T + FFN + LN2 + out, per query-half ------------
        with (
            tc.tile_pool(name="gactp", bufs=1) as gactp,
            tc.tile_pool(name="ln2p", bufs=3) as ln2p,
            tc.tile_pool(name="psTh", bufs=2, space="PSUM") as psTh,
            tc.tile_pool(name="psG", bufs=2, space="PSUM") as psG,
            tc.tile_pool(name="psF", bufs=1, space="PSUM") as psF,
        ):
            gact = gactp.tile([P, FC, 512], F16, tag="gact")

            def ht_half(qn):
                qc = slice(qn * 512, qn * 512 + 512)
                for dt in range(DC):
                    tph = psTh.tile([P, D], F16, tag="hTt", name="tph")
                    for qi in range(4):
                        qt = qn * 4 + qi
                        nc.tensor.transpose(
                            tph[:, qi * P:(qi + 1) * P],
                            h_sb[:, qt, dt * P:(dt + 1) * P], ident16[:])
                    nc.vector.tensor_copy(hT[:, dt, qc], tph[:])

            def ffn1_half(qn):
                qc = slice(qn * 512, qn * 512 + 512)
                for ft in range(FC):
                    g_ps = psG.tile([P, 512], F32, tag="g", name="g_ps")
                    for c in range(DC):
                        nc.tensor.matmul(
                            g_ps[:], lhsT=W1[:, c, ft * P:(ft + 1) * P],
                            rhs=hT[:, c, qc],
                            start=(c == 0), stop=(c == DC - 1))
                    nc.scalar.activation(gact[:, ft, :], g_ps[:], AF.Gelu,
                                         bias=b1_sb[:, ft:ft + 1])

            def ffn2_half(qn):
                # per query-tile: full-ft accumulation + residual + b2, then
                # LN2 immediately -- overlaps the next tile's matmuls.
                for qi in range(4):
                    qt = qn * 4 + qi
                    q0 = qi * P
                    ps = psF.tile([P, D], F32, tag=f"ff{qi}", name=f"ff{qi}")
                    for ft in range(FC):
                        nc.tensor.matmul(
                            ps[:], lhsT=gact[:, ft, q0:q0 + P],
                            rhs=W2[:, ft, :],
                            start=(ft == 0), stop=False)
                    nc.tensor.matmul(  # += I.T @ h: the residual add
                        ps[:], lhsT=ident16[:], rhs=h_sb[:, qt, :],
                        start=False, stop=False)
                    nc.tensor.matmul(  # += ones ⊗ b2: broadcast bias row
                        ps[:], lhsT=onesr[:], rhs=b2_sb[:],
                        start=False, stop=True)
                    stats = ln2p.tile([P, nc.vector.BN_STATS_DIM], F32,
                                      tag="st2", name="stats2")
                    nc.vector.bn_stats(out=stats[:], in_=ps[:])
                    mv = ln2p.tile([P, nc.vector.BN_AGGR_DIM], F32,
                                   tag="mv2", name="mv2")
                    nc.vector.bn_aggr(out=mv[:], in_=stats[:])
                    rstd = rsqrt_dve(ln2p, mv[:, 1:2])
                    o_t = ln2p.tile([P, D], F32, tag="o", name="o_t")
                    nc.vector.tensor_scalar(
                        out=o_t[:], in0=ps[:],
                        scalar1=mv[:, 0:1], scalar2=rstd[:, 0:1],
                        op0=ALU.subtract, op1=ALU.mult)
                    nc.sync.dma_start(out_d[qt * P:(qt + 1) * P, :], o_t[:])

            ht_half(0)
            ffn1_half(0)
            ht_half(1)
            ffn2_half(0)      # LN2 of half 0 overlaps ffn1_half(1) matmuls
            ffn1_half(1)
            ffn2_half(1)
        pW_cm.__exit__(None, None, None)
        pp_cm.__exit__(None, None, None)
    nc.finalize()
    return nc


def kernel(x, attention_mask, Wq, Wk, Wv, Wp, W1, b1, W2, b2,
           ln1_g, ln1_b, ln2_g, ln2_b):
    global last_exec_ns, last_trace_path, last_results
    x = np.asarray(x, dtype=np.float32)
    attention_mask = np.asarray(attention_mask)
    f = lambda a: np.ascontiguousarray(np.asarray(a, dtype=np.float32))
    Wq, Wk, Wv, Wp, W1, b1, W2, b2 = map(f, (Wq, Wk, Wv, Wp, W1, b1, W2, b2))
    ln1_g, ln1_b, ln2_g, ln2_b = map(f, (ln1_g, ln1_b, ln2_g, ln2_b))

    # mask compaction (exact: masked keys contribute exp(-100) ~ 0)
    idxs = [np.nonzero(attention_mask[b])[0] for b in range(B)]
    nmax = max(1, max(len(i) for i in idxs))
    NK = ((nmax + P - 1) // P) * P
    KT = NK // P

    h16 = lambda a: np.ascontiguousarray(a, dtype=np.float16)
    WqT = h16(Wq.T)
    WkT = h16(Wk.T)
    WvT = h16(Wv.T)
    WpT = h16(Wp.T)
    W1T = h16(W1.T)
    W2T = h16(W2.T)
    b1r = np.ascontiguousarray(b1.reshape(FC, P).T)
    b2r = h16(b2.reshape(1, D))

    # LN affine params are identity per the problem spec (fill: ones/zeros);
    # verify and fail loudly if that ever changes.
    assert np.all(ln1_g == 1) and np.all(ln2_g == 1), "non-identity ln gain"
    assert np.all(ln1_b == 0) and np.all(ln2_b == 0), "non-identity ln bias"

    in_maps = []
    for core in range(8):
        b, half = core // 2, core % 2
        q0 = half * NQ
        idx = idxs[b]
        nk = len(idx)
        xkv = np.zeros((NK, D), np.float32)
        xkv[:nk] = x[b][idx]
        mbias = np.full((P, KT), -100.0, np.float32)
        kk = np.arange(NK).reshape(KT, P).T  # [p, kt] -> key index
        mbias[kk < nk] = 0.0
        in_maps.append({
            "xqT": h16(x[b, q0:q0 + NQ].T),
            "xkvT": h16(xkv.T),
            "xres": h16(x[b, q0:q0 + NQ]),
            "maskbias": mbias,
            "WqT": WqT, "WkT": WkT, "WvT": WvT, "WpT": WpT,
            "W1T": W1T, "W2T": W2T, "b1r": b1r, "b2r": b2r,
        })

    if NK not in _nc_cache:
        _nc_cache[NK] = _build(NK)
    nc = _nc_cache[NK]

    trace = bool(os.environ.get("BERT_TRACE"))
    res = run_bass_kernel_spmd(nc, in_maps, core_ids=list(range(8)),
                               trace=trace)
    last_exec_ns = res.exec_time_ns
    last_results = res
    if res.instructions_and_trace:
        last_trace_path = res.instructions_and_trace[1]

    out = np.empty((B, S, D), np.float32)
    for core in range(8):
        b, half = core // 2, core % 2
        out[b, half * NQ:(half + 1) * NQ, :] = res.results[core]["out"]
    return out


# revision 26
# speedup vs baseline: 1.3165x; 1.3165x over previous
"""BERT layer kernel for 8 TRN2 NeuronCores.

Sharding: 8 cores = 4 batch elements x 2 query-halves (1024 queries each).
Each core is fully independent (no collectives): it computes attention for
its 1024 queries against its batch element's key set, then proj/LN1/FFN/LN2
for its tokens.

Key ideas:
- Mask compaction on CPU: ~50% of keys have mask=0 and contribute exp(-100)
  ~= 0; only unmasked keys (padded to a multiple of 128) are shipped/computed.
- Feature-major (transposed) activations so every matmul contracts on the
  partition dim with zero on-device transposes of x (CPU pre-transposes).
- Input DMAs issue on one FIFO HWDGE ring in strictly critical-first order
  (xkvT/Wv/Wq/xqT/Wk first, FFN weights last); the V ones-column is a
  device-side memset (padded keys contribute exp(-100) ~ 0 to the
  denominator, so all-ones is exact) -- a DMA of it would emit 9216 2-byte
  descriptors and clog the ring for ~11us.
- Scores computed key-major (sT[k, q]); softmax numerator exp(0.125*s + bias)
  runs on ScalarE directly from PSUM with the mask as a per-partition bias.
- A ones-column appended to V makes the PV matmul emit softmax denominators
  for free; each head-pair is normalized immediately after its PV completes
  (reciprocal on 2 partitions + DMA partition-broadcast + one DVE multiply),
  fully overlapped with the next pair's projections/attention.
- proj and FFN2 accumulate token-major PSUM tiles (lhsT = activations), so
  LN1/LN2 read PSUM directly: no LN transposes, no attT/ffT intermediates.
  Residual adds (x, h) and the b2 row ride the same PSUM accumulation as
  identity / ones-outer-product matmuls.
- LN rsqrt(var+eps) via fast-inverse-sqrt (bit trick + 1 Newton step) on the
  DVE: keeps SQRT off ScalarE so only EXP and GELU activation tables load
  (table switches cost ~1.3us each and serialize the ACT pipeline).
- FFN2 runs per query-tile (gact for the whole half staged in SBUF), so each
  tile's LN2 overlaps the next tile's matmuls; only the last tile's chain is
  exposed.
- All matmuls fp16 (full PE rate, ~1.5e-4 rounding).
"""
import os
import sys

for _p in ("/opt/trn_rl_repo", "/root/.axon_site/_ro/trn_rl_repo"):
    if os.path.isdir(_p) and _p not in sys.path:
        sys.path.append(_p)

import numpy as np
import concourse.bacc as bacc
import concourse.tile as tile
from concourse import mybir
from concourse.bass_utils import run_bass_kernel_spmd
from concourse.masks import make_identity

P = 128
B, S, D = 4, 2048, 512
H, DK, DV = 8, 64, 64
DFF = 2048
NQ = 1024          # queries per core
QT_TILES = NQ // P  # 8
DC = D // P         # 4 feature chunks
FC = DFF // P       # 16 ffn chunks
LN_EPS = 1e-5
RSQRT_MAGIC = 0x5F3759DF

F32 = mybir.dt.float32
F16 = mybir.dt.float16
I32 = mybir.dt.int32
AF = mybir.ActivationFunctionType
ALU = mybir.AluOpType

_nc_cache = {}
last_exec_ns = None
last_trace_path = None
last_results = None

# rsqrt: 'dve' = fast-inverse-sqrt bit trick; 'scalar' = ACT sqrt + DVE recip
RSQRT_MODE = os.environ.get("BERT_RSQRT_MODE", "dve")


def _build(NK):
    """Build the per-core Bass program for NK (padded, multiple of 128) keys."""
    KT = NK // P
    nc = bacc.Bacc(None, target_bir_lowering=False)

    # ---- DRAM I/O ----
    # All big inputs are pre-arranged on the host to the exact SBUF layout
    # [P, chunks, free] so every DMA is 128 contiguous multi-KB lines
    # (strided 2-4KB descriptor lines top out around ~150 GB/s).
    xqT_d = nc.dram_tensor("xqT", [P, DC, NQ], F16, kind="ExternalInput")
    xkvT_d = nc.dram_tensor("xkvT", [P, DC, NK], F16, kind="ExternalInput")
    xres_d = nc.dram_tensor("xres", [P, QT_TILES, D], F16,
                            kind="ExternalInput")
    mb_d = nc.dram_tensor("maskbias", [P, KT], F32, kind="ExternalInput")
    WqT_d = nc.dram_tensor("WqT", [P, DC, D], F16, kind="ExternalInput")
    WkT_d = nc.dram_tensor("WkT", [P, DC, D], F16, kind="ExternalInput")
    WvT_d = nc.dram_tensor("WvT", [P, DC, D], F16, kind="ExternalInput")
    WpT_d = nc.dram_tensor("WpT", [P, DC, D], F16, kind="ExternalInput")
    W1T_d = nc.dram_tensor("W1T", [P, DC, DFF], F16, kind="ExternalInput")
    W2T_d = nc.dram_tensor("W2T", [P, FC, D], F16, kind="ExternalInput")
    b1_d = nc.dram_tensor("b1r", [P, FC], F32, kind="ExternalInput")
    b2_d = nc.dram_tensor("b2r", [1, D], F16, kind="ExternalInput")
    out_d = nc.dram_tensor("out", [NQ, D], F32, kind="ExternalOutput")

    with tile.TileContext(nc) as tc:
        # Pools close LIFO; opened in reverse order of tensor death.
        pp_cm = tc.tile_pool(name="pp", bufs=1)
        pAB_cm = tc.tile_pool(name="pAB", bufs=1)   # ctxT + WpT
        pA_cm = tc.tile_pool(name="pA", bufs=1)     # QT/KTs/Vs
        ph1_cm = tc.tile_pool(name="ph1", bufs=1)   # xqT/xkvT/Wq/Wk/Wv
        pW_cm = tc.tile_pool(name="pW", bufs=1, side="right")
        pp = pp_cm.__enter__()
        pW = pW_cm.__enter__()
        pAB = pAB_cm.__enter__()
        pA = pA_cm.__enter__()
        ph1 = ph1_cm.__enter__()

        # tiles (declared in dependency-phase order)
        mb_sb = pp.tile([P, KT], F32, tag="mb")
        b1_sb = pp.tile([P, FC], F32, tag="b1")
        b2_sb = pp.tile([1, D], F16, tag="b2")
        magic = pp.tile([P, 1], I32, tag="magic")
        onesr = pp.tile([1, P], F16, tag="onesr")
        eps_sb = pp.tile([P, 1], F32, tag="eps")
        ident = pp.tile([P, P], F32, tag="ident")
        ident16 = pp.tile([P, P], F16, tag="ident16")
        WpT = pAB.tile([P, DC, D], F16, tag="WpT")
        QT = pA.tile([P, DC, NQ], F16, tag="QT")
        KTs = pA.tile([P, DC, NK], F16, tag="KTs")
        Vs = pA.tile([P, KT, H, DV + 1], F16, tag="Vs")
        ctxT = pAB.tile([P, DC, NQ], F16, tag="ctxT")
        xqT = ph1.tile([P, DC, NQ], F16, tag="xqT")
        xkvT = ph1.tile([P, DC, NK], F16, tag="xkvT")
        Wq = ph1.tile([P, DC, D], F16, tag="Wq")
        Wk = ph1.tile([P, DC, D], F16, tag="Wk")
        Wv = ph1.tile([P, DC, D], F16, tag="Wv")
        W1 = pW.tile([P, DC, DFF], F16, tag="W1")
        W2 = pW.tile([P, FC, D], F16, tag="W2")
        xres = pW.tile([P, QT_TILES, D], F16, tag="xres")
        h_sb = pp.tile([P, QT_TILES, D], F16, tag="h_sb")
        hT = pp.tile([P, DC, NQ], F16, tag="hT")

        # input DMAs: one FIFO ring, critical-first
        nc.sync.dma_start(mb_sb[:], mb_d[:])
        nc.sync.dma_start(xkvT[:], xkvT_d[:])
        nc.sync.dma_start(Wv[:], WvT_d[:])
        nc.sync.dma_start(Wq[:], WqT_d[:])
        nc.sync.dma_start(xqT[:], xqT_d[:])
        nc.sync.dma_start(Wk[:], WkT_d[:])
        nc.sync.dma_start(WpT[:], WpT_d[:])
        nc.sync.dma_start(b1_sb[:], b1_d[:])
        nc.sync.dma_start(b2_sb[:], b2_d[:])
        nc.sync.dma_start(W1[:], W1T_d[:])
        nc.sync.dma_start(W2[:], W2T_d[:])
        nc.sync.dma_start(xres[:], xres_d[:])

        nc.vector.memset(Vs[:, :, :, DV], 1.0)  # softmax-denominator column
        nc.vector.memset(magic[:], RSQRT_MAGIC)
        nc.vector.memset(onesr[:], 1.0)
        nc.vector.memset(eps_sb[:], LN_EPS)
        make_identity(nc, ident[:])
        nc.vector.tensor_copy(ident16[:], ident[:])

        kchunks = ([(i * 384, 384) for i in range(NK // 384)]
                   if NK % 384 == 0 else
                   [(s0, min(512, NK - s0)) for s0 in range(0, NK, 512)])
        with (
            tc.tile_pool(name="ps1", bufs=2, space="PSUM") as ps1,
            tc.tile_pool(name="epool", bufs=3) as epool,
            tc.tile_pool(name="denp", bufs=2) as denp,
            tc.tile_pool(name="psS", bufs=2, space="PSUM") as psS,
            tc.tile_pool(name="psC", bufs=1, space="PSUM") as psC,
        ):
            # V[k, dv] token-major (+ ones col memset above) -- feeds all pairs
            for kt in range(KT):
                ps = ps1.tile([P, D], F32, tag="p1", name="psv")
                for c in range(DC):
                    nc.tensor.matmul(
                        ps[:], lhsT=xkvT[:, c, kt * P:(kt + 1) * P],
                        rhs=Wv[:, c, :],
                        start=(c == 0), stop=(c == DC - 1))
                nc.vector.tensor_copy(
                    Vs[:, kt, :, 0:DV],
                    ps.rearrange("p (h v) -> p h v", h=H))
            def rb_and_mul(cc, rcp_row):
                """Normalize pair cc's ctxT: ones-outer-product broadcast of
                1/den on the PE, then DVE multiply straight from PSUM.  Runs
                one pair late in the PE stream so the reciprocal chain is
                long done and never stalls the PE."""
                for qn in range(2):
                    qs = slice(qn * 512, (qn + 1) * 512)
                    rp = ps1.tile([P, 512], F32, tag="p1", name="rb_ps")
                    nc.tensor.matmul(
                        rp[0:64, :], lhsT=onesr[0:1, 0:64],
                        rhs=rcp_row[0:1, (qn * 2) * 512:(qn * 2 + 1) * 512],
                        start=True, stop=True)
                    nc.tensor.matmul(
                        rp[64:128, :], lhsT=onesr[0:1, 0:64],
                        rhs=rcp_row[0:1, (qn * 2 + 1) * 512:(qn * 2 + 2) * 512],
                        start=True, stop=True)
                    nc.vector.tensor_mul(ctxT[:, cc, qs], ctxT[:, cc, qs],
                                         rp[:])

            # per head-pair: project QT/KT chunk, then attention for the pair.
            # The dense K=128 projection matmuls of pair c+1 fill the PE while
            # ScalarE chews pair c's exps (keeps HAM warm).
            pending = None
            for c in range(DC):
                ha, hb = 2 * c, 2 * c + 1
                for qn in range(NQ // 512):
                    ps = ps1.tile([P, 512], F32, tag="p1", name="psq")
                    for cc in range(DC):
                        nc.tensor.matmul(
                            ps[:],
                            lhsT=Wq[:, cc, c * P:(c + 1) * P],
                            rhs=xqT[:, cc, qn * 512:(qn + 1) * 512],
                            start=(cc == 0), stop=(cc == DC - 1))
                    nc.vector.tensor_copy(
                        QT[:, c, qn * 512:(qn + 1) * 512], ps[:])
                for (s0, w) in kchunks:
                    ps = ps1.tile([P, 512], F32, tag="p1", name="psk")
                    for cc in range(DC):
                        nc.tensor.matmul(
                            ps[:, 0:w],
                            lhsT=Wk[:, cc, c * P:(c + 1) * P],
                            rhs=xkvT[:, cc, s0:s0 + w],
                            start=(cc == 0), stop=(cc == DC - 1))
                    nc.vector.tensor_copy(KTs[:, c, s0:s0 + w], ps[:, 0:w])
                if pending is not None:
                    rb_and_mul(*pending)
                # attention for heads (ha, hb): row-packed K=64 matmuls into
                # one shared psum tile (a: cols 0:512, b: 512:1024)
                # den rows staged [1, 4*512] in (qn, head) section order
                den_row = denp.tile([1, 4 * 512], F32, tag="den", name="den")
                for qn in range(2):
                    ctxa = psC.tile([P, 512], F32, tag="ctxa", name="ctxa")
                    ctxb = psC.tile([P, 512], F32, tag="ctxb", name="ctxb")
                    for kt in range(KT):
                        sp = psS.tile([P, 1024], F32, tag="sT", name="sp")
                        nc.tensor.matmul(
                            sp[:, 0:512],
                            lhsT=KTs[0:64, c, kt * P:(kt + 1) * P],
                            rhs=QT[0:64, c, qn * 512:(qn + 1) * 512],
                            start=True, stop=True)
                        nc.tensor.matmul(
                            sp[:, 512:1024],
                            lhsT=KTs[64:128, c, kt * P:(kt + 1) * P],
                            rhs=QT[64:128, c, qn * 512:(qn + 1) * 512],
                            start=True, stop=True)
                        e_t = epool.tile([P, 1024], F16, tag="E", name="e_t")
                        nc.scalar.activation(e_t[:], sp[:], AF.Exp,
                                             bias=mb_sb[:, kt:kt + 1],
                                             scale=float(DK) ** -0.5)
                        nc.tensor.matmul(
                            ctxa[0:DV + 1, :], lhsT=Vs[:, kt, ha, :],
                            rhs=e_t[:, 0:512],
                            start=(kt == 0), stop=(kt == KT - 1))
                        nc.tensor.matmul(
                            ctxb[0:DV + 1, :], lhsT=Vs[:, kt, hb, :],
                            rhs=e_t[:, 512:1024],
                            start=(kt == 0), stop=(kt == KT - 1))
                    for (i, cx) in ((0, ctxa), (1, ctxb)):
                        sec = qn * 2 + i
                        nc.vector.tensor_copy(
                            den_row[0:1, sec * 512:(sec + 1) * 512],
                            cx[DV:DV + 1, :])
                        nc.vector.tensor_copy(
                            ctxT[i * 64:i * 64 + 64, c,
                                 qn * 512:(qn + 1) * 512],
                            cx[0:DV, :])
                # reciprocal chain: a [1, 2048] row is 1-lane work on the DVE
                # (~6 cyc/elem), so restripe to [128, 16] by DMA, recip there
                # (~0.2us), and destripe back for the matmul broadcast.
                den_st = denp.tile([P, 16], F32, tag="dst", name="den_st")
                nc.gpsimd.dma_start(den_st[:], den_row[:])
                rcp_st = denp.tile([P, 16], F16, tag="rst", name="rcp_st")
                with nc.allow_low_precision(reason="fp16 softmax recip"):
                    nc.vector.reciprocal(rcp_st[:], den_st[:])
                rcp_row = denp.tile([1, 4 * 512], F16, tag="rcp", name="rcp")
                nc.gpsimd.dma_start(rcp_row[:], rcp_st[:])
                pending = (c, rcp_row)
            rb_and_mul(*pending)
        ph1_cm.__exit__(None, None, None)
        pA_cm.__exit__(None, None, None)  # QT/KTs/Vs dead

        # ---------------- proj + LN1 (token-major, from PSUM) --------------
        def rsqrt_dve(pool, var_ap):
            """rstd = 1/sqrt(var+eps), fast-inverse-sqrt on DVE (no ACT
            table): y0 = bits(magic - bits(v)>>1); 1 Newton step."""
            if RSQRT_MODE == "scalar":
                w = pool.tile([P, 1], F32, tag="w", name="w")
                nc.scalar.activation(out=w[:], in_=var_ap, func=AF.Sqrt,
                                     bias=eps_sb[:, 0:1])
                nc.vector.reciprocal(out=w[:], in_=w[:])
                return w
            v = pool.tile([P, 1], F32, tag="v", name="v")
            nc.vector.tensor_scalar(out=v[:], in0=var_ap, scalar1=LN_EPS,
                                    scalar2=None, op0=ALU.add)
            y0 = pool.tile([P, 1], I32, tag="y0", name="y0")
            nc.vector.tensor_scalar(out=y0[:], in0=v.bitcast(I32),
                                    scalar1=1, scalar2=None,
                                    op0=ALU.logical_shift_right)
            nc.vector.tensor_tensor(out=y0[:], in0=magic[:], in1=y0[:],
                                    op=ALU.subtract)
            y0f = y0.bitcast(F32)
            w = pool.tile([P, 1], F32, tag="w", name="w")
            nc.vector.tensor_tensor(out=w[:], in0=y0f, in1=y0f, op=ALU.mult)
            nc.vector.tensor_scalar(out=w[:], in0=w[:], scalar1=v[:, 0:1],
                                    scalar2=-0.5, op0=ALU.mult, op1=ALU.mult)
            nc.vector.tensor_scalar(out=w[:], in0=w[:], scalar1=1.5,
                                    scalar2=None, op0=ALU.add)
            nc.vector.tensor_tensor(out=w[:], in0=y0f, in1=w[:], op=ALU.mult)
            return w

        with (
            tc.tile_pool(name="psP", bufs=2, space="PSUM") as psP,
            tc.tile_pool(name="ln1p", bufs=3, side="right") as ln1p,
        ):
            for qt in range(QT_TILES):
                ps = psP.tile([P, D], F32, tag="att", name="att_ps")
                for c in range(DC):
                    nc.tensor.matmul(
                        ps[:], lhsT=ctxT[:, c, qt * P:(qt + 1) * P],
                        rhs=WpT[:, c, :],
                        start=(c == 0), stop=False)
                nc.tensor.matmul(  # += I.T @ xres: the residual add
                    ps[:], lhsT=ident16[:], rhs=xres[:, qt, :],
                    start=False, stop=True)
                stats = ln1p.tile([P, nc.vector.BN_STATS_DIM], F32,
                                  tag="st1", name="stats")
                nc.vector.bn_stats(out=stats[:], in_=ps[:])
                mv = ln1p.tile([P, nc.vector.BN_AGGR_DIM], F32, tag="mv1",
                               name="mv")
                nc.vector.bn_aggr(out=mv[:], in_=stats[:])
                rstd = rsqrt_dve(ln1p, mv[:, 1:2])
                nc.vector.tensor_scalar(
                    out=h_sb[:, qt, :], in0=ps[:],
                    scalar1=mv[:, 0:1], scalar2=rstd[:, 0:1],
                    op0=ALU.subtract, op1=ALU.mult)
        pAB_cm.__exit__(None, None, None)  # ctxT, WpT dead

        # ---------------- h^T + FFN + LN2 + out, per query-half ------------
        with (
            tc.tile_pool(name="gactp", bufs=1) as gactp,
            tc.tile_pool(name="ln2p", bufs=3) as ln2p,
            tc.tile_pool(name="psTh", bufs=2, space="PSUM") as psTh,
            tc.tile_pool(name="psG", bufs=2, space="PSUM") as psG,
            tc.tile_pool(name="psF", bufs=1, space="PSUM") as psF,
        ):
            gact = gactp.tile([P, FC, 512], F16, tag="gact")

            def ht_half(qn):
                qc = slice(qn * 512, qn * 512 + 512)
                for dt in range(DC):
                    tph = psTh.tile([P, D], F16, tag="hTt", name="tph")
                    for qi in range(4):
                        qt = qn * 4 + qi
                        nc.tensor.transpose(
                            tph[:, qi * P:(qi + 1) * P],
                            h_sb[:, qt, dt * P:(dt + 1) * P], ident16[:])
                    nc.vector.tensor_copy(hT[:, dt, qc], tph[:])

            def ffn1_half(qn):
                qc = slice(qn * 512, qn * 512 + 512)
                for ft in range(FC):
                    g_ps = psG.tile([P, 512], F32, tag="g", name="g_ps")
                    for c in range(DC):
                        nc.tensor.matmul(
                            g_ps[:], lhsT=W1[:, c, ft * P:(ft + 1) * P],
                            rhs=hT[:, c, qc],
                            start=(c == 0), stop=(c == DC - 1))
                    nc.scalar.activation(gact[:, ft, :], g_ps[:], AF.Gelu,
                                         bias=b1_sb[:, ft:ft + 1])

            def ffn2_half(qn):
                # per query-tile: full-ft accumulation + residual + b2, then
                # LN2 immediately -- overlaps the next tile's matmuls.
                for qi in range(4):
                    qt = qn * 4 + qi
                    q0 = qi * P
                    ps = psF.tile([P, D], F32, tag=f"ff{qi}", name=f"ff{qi}")
                    for ft in range(FC):
                        nc.tensor.matmul(
                            ps[:], lhsT=gact[:, ft, q0:q0 + P],
                            rhs=W2[:, ft, :],
                            start=(ft == 0), stop=False)
                    nc.tensor.matmul(  # += I.T @ h: the residual add
                        ps[:], lhsT=ident16[:], rhs=h_sb[:, qt, :],
                        start=False, stop=False)
                    nc.tensor.matmul(  # += ones ⊗ b2: broadcast bias row
                        ps[:], lhsT=onesr[:], rhs=b2_sb[:],
                        start=False, stop=True)
                    stats = ln2p.tile([P, nc.vector.BN_STATS_DIM], F32,
                                      tag="st2", name="stats2")
                    nc.vector.bn_stats(out=stats[:], in_=ps[:])
                    mv = ln2p.tile([P, nc.vector.BN_AGGR_DIM], F32,
                                   tag="mv2", name="mv2")
                    nc.vector.bn_aggr(out=mv[:], in_=stats[:])
                    rstd = rsqrt_dve(ln2p, mv[:, 1:2])
                    o_t = ln2p.tile([P, D], F32, tag="o", name="o_t")
                    nc.vector.tensor_scalar(
                        out=o_t[:], in0=ps[:],
                        scalar1=mv[:, 0:1], scalar2=rstd[:, 0:1],
                        op0=ALU.subtract, op1=ALU.mult)
                    nc.sync.dma_start(out_d[qt * P:(qt + 1) * P, :], o_t[:])

            ht_half(0)
            ffn1_half(0)
            ht_half(1)
            ffn2_half(0)      # LN2 of half 0 overlaps ffn1_half(1) matmuls
            ffn1_half(1)
            ffn2_half(1)
        pW_cm.__exit__(None, None, None)
        pp_cm.__exit__(None, None, None)
    nc.finalize()
    return nc


def kernel(x, attention_mask, Wq, Wk, Wv, Wp, W1, b1, W2, b2,
           ln1_g, ln1_b, ln2_g, ln2_b):
    global last_exec_ns, last_trace_path, last_results
    x = np.asarray(x, dtype=np.float32)
    attention_mask = np.asarray(attention_mask)
    f = lambda a: np.ascontiguousarray(np.asarray(a, dtype=np.float32))
    Wq, Wk, Wv, Wp, W1, b1, W2, b2 = map(f, (Wq, Wk, Wv, Wp, W1, b1, W2, b2))
    ln1_g, ln1_b, ln2_g, ln2_b = map(f, (ln1_g, ln1_b, ln2_g, ln2_b))

    # mask compaction (exact: masked keys contribute exp(-100) ~ 0)
    idxs = [np.nonzero(attention_mask[b])[0] for b in range(B)]
    nmax = max(1, max(len(i) for i in idxs))
    NK = ((nmax + P - 1) // P) * P
    KT = NK // P

    h16 = lambda a: np.ascontiguousarray(a, dtype=np.float16)
    # chunked([R, C], n) -> [P, n, C]: partition-major SBUF image, so each
    # input DMA is 128 contiguous multi-KB lines.
    chunked = lambda a, n: np.ascontiguousarray(
        np.asarray(a, dtype=np.float16).reshape(n, P, -1).transpose(1, 0, 2))
    WqT = chunked(Wq.T, DC)
    WkT = chunked(Wk.T, DC)
    WvT = chunked(Wv.T, DC)
    WpT = chunked(Wp.T, DC)
    W1T = chunked(W1.T, DC)
    W2T = chunked(W2.T, FC)
    b1r = np.ascontiguousarray(b1.reshape(FC, P).T)
    b2r = h16(b2.reshape(1, D))

    # LN affine params are identity per the problem spec (fill: ones/zeros);
    # verify and fail loudly if that ever changes.
    assert np.all(ln1_g == 1) and np.all(ln2_g == 1), "non-identity ln gain"
    assert np.all(ln1_b == 0) and np.all(ln2_b == 0), "non-identity ln bias"

    in_maps = []
    for core in range(8):
        b, half = core // 2, core % 2
        q0 = half * NQ
        idx = idxs[b]
        nk = len(idx)
        xkv = np.zeros((NK, D), np.float32)
        xkv[:nk] = x[b][idx]
        mbias = np.full((P, KT), -100.0, np.float32)
        kk = np.arange(NK).reshape(KT, P).T  # [p, kt] -> key index
        mbias[kk < nk] = 0.0
        in_maps.append({
            "xqT": chunked(x[b, q0:q0 + NQ].T, DC),
            "xkvT": chunked(xkv.T, DC),
            "xres": chunked(x[b, q0:q0 + NQ], QT_TILES),
            "maskbias": mbias,
            "WqT": WqT, "WkT": WkT, "WvT": WvT, "WpT": WpT,
            "W1T": W1T, "W2T": W2T, "b1r": b1r, "b2r": b2r,
        })

    if NK not in _nc_cache:
        _nc_cache[NK] = _build(NK)
    nc = _nc_cache[NK]

    trace = bool(os.environ.get("BERT_TRACE"))
    res = run_bass_kernel_spmd(nc, in_maps, core_ids=list(range(8)),
                               trace=trace)
    last_exec_ns = res.exec_time_ns
    last_results = res
    if res.instructions_and_trace:
        last_trace_path = res.instructions_and_trace[1]

    out = np.empty((B, S, D), np.float32)
    for core in range(8):
        b, half = core // 2, core % 2
        out[b, half * NQ:(half + 1) * NQ, :] = res.results[core]["out"]
    return out
